# revision 1
# baseline (speedup 1.0000x reference)
"""Multi-head causal attention on 8 Trainium2 NeuronCores — v2.

Sharding: tensor-parallel over heads (2 heads/core) for QKV + attention;
per-(batch, q-tile) AllToAlls convert to token-sharding (tokens interleaved
at 64-token granularity) so the collectives pipeline behind attention
compute, and the output projection for each q-tile runs as soon as its two
A2As land. The host gather is pure concatenation (interleaved slices).

Per-core layout:
  - everything bf16 on the wire (x, Wqkv, Wo, A2A payloads); f32 PSUM accum.
  - qkvT = W^T x^T in [feature, token] layout; V additionally PE-transposed
    to [token, feature] for the AV matmuls, with a ones-column appended so
    the softmax denominator falls out of the AV accumulation (row 64).
  - softmax: exp on ScalarE with 1/sqrt(D) folded into the activation
    scale; no max-subtraction (scores are O(6)); causal masking is a
    multiplicative 0/1 bf16 mask on diagonal tiles (DVE).
  - normalize: 1/denom = exp(-ln(denom)) on ScalarE (DVE reciprocal is
    ~3.3us/op; ACT ln+exp is ~10x cheaper), broadcast across partitions
    with a PE outer product into the unused upper half of the AV PSUM
    bank, single DVE multiply writes the bf16 A2A payload.
"""

import numpy as np
import ml_dtypes

import concourse.bass as bass
import concourse.mybir as mybir
import concourse.tile as tile
from concourse.bass_utils import run_bass_kernel_spmd
from concourse.masks import make_identity
from concourse.vector_clock import ScopedClock

F32 = mybir.dt.float32
F32R = mybir.dt.float32r
BF16 = mybir.dt.bfloat16
AF = mybir.ActivationFunctionType


def _install_cache_nonce_hook():
    """The libneuronxla NEFF cache hashes the HLO but the BIR rides in
    backend_config (excluded from the hash); inject a hash of the BIR into
    mhlo.frontend_attributes which IS part of the model hash."""
    import hashlib
    import concourse.bass2jax as bass2jax
    from jax.interpreters import mlir

    if getattr(bass2jax, "_ant_cache_nonce_hooked", False):
        return
    bass2jax._ant_cache_nonce_hooked = True
    orig = bass2jax._accumulate_module_dve_attrs

    def patched(ctx, nc):
        orig(ctx, nc)
        op = ctx.module_context.module.operation
        cur = (
            op.attributes["mhlo.frontend_attributes"]
            if "mhlo.frontend_attributes" in op.attributes
            else None
        )
        existing = (
            {a.name: mlir.ir.StringAttr(a.attr).value for a in cur}
            if cur is not None
            else {}
        )
        existing["ant.cache_nonce"] = hashlib.sha256(
            nc.to_json_bytes()
        ).hexdigest()
        op.attributes["mhlo.frontend_attributes"] = mlir.ir.DictAttr.get(
            {k: mlir.ir.StringAttr.get(v) for k, v in existing.items()}
        )

    bass2jax._accumulate_module_dve_attrs = patched


_install_cache_nonce_hook()


def _install_ldw_opt_hook():
    """bass_utils hardcodes --enable-ldw-opt=false; with it, walrus emits a
    serialized LDWEIGHTS before every MATMUL (~40% PE overhead here). The
    codegen with ldw-opt=true is broken for fp32 stationary weights, but all
    matmul stationaries in this kernel are bf16."""
    import concourse.bass_utils as bu

    if getattr(bu, "_ant_ldw_opt_hooked", False):
        return
    bu._ant_ldw_opt_hooked = True
    orig = bu.run_command

    def patched(argv, **kwargs):
        argv = [
            "--enable-ldw-opt=true" if a == "--enable-ldw-opt=false" else a
            for a in argv
        ]
        return orig(argv, **kwargs)

    bu.run_command = patched


# _install_ldw_opt_hook()  # walrus in this container cannot codegen ldw-opt


B, S, DM = 2, 2048, 1024
H, D = 16, 64
NCORES = 8
HP = H // NCORES          # heads per core
T = B * S                 # 4096 tokens
NCH = 8                   # token chunks of 512 (b*4 + qt)
KT_PER_S = S // 128       # 16 k-tiles per sequence
QT_PER_S = S // 512       # 4 q-tiles per sequence
SCALE = 1.0 / np.sqrt(D)

MAX_WAITS = 1  # walrus in this container rejects >1 sem-wait per instruction


def _split_waits(nc, limit=MAX_WAITS):
    """Post-pass: move excess sem-waits onto preceding same-engine nops."""
    n_id = 0
    for bb in nc.main_func.blocks:
        new = []
        for inst in bb.instructions:
            si = getattr(inst, "sync_info", None)
            if si is not None and len(si.on_wait) > limit:
                waits = list(si.on_wait)
                for i in range(0, len(waits) - limit, limit):
                    nop = mybir.InstNoOp(
                        name=f"wsplit-{n_id}", ins=[], outs=[], engine=inst.engine
                    )
                    n_id += 1
                    nop.sync_info = mybir.SyncInfo(
                        on_wait=waits[i : i + limit], on_update=[]
                    )
                    new.append(nop)
                kept = waits[len(waits) - limit :]
                inst.sync_info = mybir.SyncInfo(
                    on_wait=kept, on_update=list(si.on_update)
                )
            new.append(inst)
        bb.instructions = new


class _TileCtx(tile.TileContext):
    """Split the tail drain's multi-waits (this walrus build rejects >1-2
    sem-waits per instruction)."""

    def _drain_and_barrier(self, tick_clock, wait_clock):
        nc = self.nc
        drain_inst = nc.sync.drain()
        wait_clock.add_sem_waits(
            drain_inst.ins, ScopedClock({None: tick_clock.global_clock})
        )
        si = drain_inst.ins.sync_info
        if si is not None and len(si.on_wait) > 1:
            waits = list(si.on_wait)
            drain_inst.ins.sync_info = mybir.SyncInfo(
                on_wait=[waits[0]], on_update=list(si.on_update)
            )
            for w in waits[1:]:
                nop = nc.sync.nop(nofuse=True, hint="tail_drain_wait_split")
                nop.ins.sync_info = mybir.SyncInfo(on_wait=[w], on_update=[])

        nc.all_engine_barrier()
        assert self.sems is not None
        popped = nc._tile_sem_poison_stack.pop()
        assert popped is self._sem_poison
        nc.clear_and_free_semaphores(list(self.sems.allocated().values()))
        nc.all_engine_barrier()


def _nkt(qt, mode):
    """Number of k-tiles attended by q-tile qt (within one sequence)."""
    return 4 * (qt + 1) if mode == "causal" else KT_PER_S


def build(mode, n_mask_tiles, debug_stage=None):
    """Build the SPMD Bass program. mode: 'causal' | 'full' | 'general'."""
    nc = bass.Bass()

    xT = nc.dram_tensor("xT", [DM, T], BF16, kind="ExternalInput")
    wq = nc.dram_tensor("wq", [DM, 128], BF16, kind="ExternalInput")
    wk = nc.dram_tensor("wk", [DM, 128], BF16, kind="ExternalInput")
    wv = nc.dram_tensor("wv", [DM, 128], BF16, kind="ExternalInput")
    wo = nc.dram_tensor("wo", [DM, DM], BF16, kind="ExternalInput")
    if n_mask_tiles:
        mt = nc.dram_tensor(
            "mt", [n_mask_tiles, 128, 512], BF16, kind="ExternalInput"
        )
    # out rows: [qt][64 tokens of b0 | 64 tokens of b1]; this core's token
    # slice of q-tile qt is [512*qt + 64*core_id, +64) in each batch.
    out = nc.dram_tensor("out", [4 * 128, DM], F32, kind="ExternalOutput")

    with _TileCtx(nc) as tc:
        with (
            tc.tile_pool(name="const", bufs=1) as const,
            tc.tile_pool(
                name="xin", bufs=8 if n_mask_tiles <= 4 else 3
            ) as xin,
            tc.tile_pool(name="stage", bufs=3) as stage,
            tc.tile_pool(name="pp", bufs=8) as pp,
            tc.tile_pool(name="misc", bufs=4) as misc,
            tc.tile_pool(name="ps_misc", bufs=2, space="PSUM") as ps_misc,
            tc.tile_pool(name="ps_av", bufs=2, space="PSUM") as ps_av,
            tc.tile_pool(name="ps1024", bufs=2, space="PSUM") as ps1024,
            tc.tile_pool(name="dram", bufs=1, space="DRAM") as dram,
        ):
            import contextlib

            _stk = contextlib.ExitStack()
            dramp = [
                _stk.enter_context(
                    tc.tile_pool(name=f"dram{ch}", bufs=1, space="DRAM")
                )
                for ch in range(NCH)
            ]
            # ---- resident SBUF tensors ----
            wq_sb = const.tile([128, 8, 128], BF16)
            wk_sb = const.tile([128, 8, 128], BF16)
            wv_sb = const.tile([128, 8, 128], BF16)
            nc.sync.dma_start(wq_sb[:], wq.rearrange("(o p) e -> p o e", p=128))
            xt0 = xin.tile([128, 8, 512], BF16, tag="xt", name="xt0")
            for hf in range(2):
                nc.sync.dma_start(
                    xt0[:, 4 * hf : 4 * (hf + 1), :],
                    xT[512 * hf : 512 * (hf + 1), 0:512].rearrange(
                        "(o p) s -> p o s", p=128
                    ),
                )
            nc.sync.dma_start(wk_sb[:], wk.rearrange("(o p) e -> p o e", p=128))
            nc.sync.dma_start(wv_sb[:], wv.rearrange("(o p) e -> p o e", p=128))
            if n_mask_tiles:
                mt_sb = const.tile([128, n_mask_tiles, 512], BF16)
                nc.sync.dma_start(mt_sb[:], mt.rearrange("m p q -> p m q"))
            xts = {0: xt0}
            if n_mask_tiles <= 4:
                # causal/full: keep all of x resident (fits SBUF); the
                # general-mask build trades this for the 64-tile mask.
                for c in (4, 1, 5, 2, 6, 3, 7):
                    xtc = xin.tile(
                        [128, 8, 512], BF16, tag="xt", name=f"xt{c}"
                    )
                    nc.sync.dma_start(
                        xtc[:],
                        xT[:, 512 * c : 512 * (c + 1)].rearrange(
                            "(o p) s -> p o s", p=128
                        ),
                    )
                    xts[c] = xtc
            wo_sb = const.tile([128, 8, DM], BF16)

            qT_sb = const.tile([128, NCH, 512], BF16)
            kT_sb = const.tile([128, NCH, 512], BF16)
            # V in [token, feature] layout, per k-tile, per head:
            # [p=token%128, ktile, head, 80] cols 0:64 = v, col 64 = 1.0
            v_sb = const.tile([128, T // 128, HP, 80], BF16)
            nc.vector.memset(v_sb[:, :, :, 64:65], 1.0)
            ident = const.tile([128, 128], F32)
            make_identity(nc, ident[:])
            ones64 = const.tile([1, 64], BF16)
            nc.vector.memset(ones64[:], 1.0)

            warm_in = dram.tile([NCORES, 1, 2], BF16, name="warm_in")
            warm_out = dram.tile([NCORES, 1, 2], BF16, name="warm_out")
            nc.gpsimd.collective_compute(
                "AllToAll",
                mybir.AluOpType.bypass,
                replica_groups=[list(range(NCORES))],
                ins=[warm_in.opt()],
                outs=[warm_out.opt()],
            )
            a2a_in = [
                dramp[ch].tile([NCORES, 128, 64], BF16, name=f"a2a_in{ch}")
                for ch in range(NCH)
            ]
            a2a_out = [
                dramp[ch].tile([NCORES, 128, 64], BF16, name=f"a2a_out{ch}")
                for ch in range(NCH)
            ]

            def qkv_chunk(c):
                if c in xts:
                    xt = xts[c]
                else:
                    xt = xin.tile(
                        [128, 8, 512], BF16, tag="xt", name=f"xt{c}"
                    )
                    nc.sync.dma_start(
                        xt[:],
                        xT[:, 512 * c : 512 * (c + 1)].rearrange(
                            "(o p) s -> p o s", p=128
                        ),
                    )
                for name, w_sb, dst in (
                    ("q", wq_sb, qT_sb),
                    ("k", wk_sb, kT_sb),
                ):
                    psum = ps_misc.tile(
                        [128, 512], F32, tag="psm", name=f"ps_{name}{c}"
                    )
                    for kt in range(8):
                        nc.tensor.matmul(
                            psum[:],
                            w_sb[:, kt, :],
                            xt[:, kt, :],
                            start=(kt == 0),
                            stop=(kt == 7),
                        )
                    nc.vector.tensor_copy(dst[:, c, :], psum[:])
                psum = ps_misc.tile([128, 512], F32, tag="psm", name=f"ps_v{c}")
                for kt in range(8):
                    nc.tensor.matmul(
                        psum[:],
                        wv_sb[:, kt, :],
                        xt[:, kt, :],
                        start=(kt == 0),
                        stop=(kt == 7),
                    )
                vstg = stage.tile([128, 512], F32, tag="vstg")
                nc.vector.tensor_copy(vstg[:], psum[:])
                ps_t = ps_misc.tile([128, 512], F32, tag="psm", name=f"ps_t{c}")
                for sub in range(4):
                    nc.tensor.transpose(
                        ps_t[:, 128 * sub : 128 * (sub + 1)],
                        vstg[:, 128 * sub : 128 * (sub + 1)],
                        ident[:],
                    )
                for sub in range(4):
                    ktile = 4 * c + sub
                    nc.vector.tensor_copy(
                        v_sb[:, ktile, :, 0:64],
                        ps_t[:, 128 * sub : 128 * (sub + 1)].rearrange(
                            "p (h d) -> p h d", h=HP
                        ),
                    )

            def attention(b, qt):
                ch = b * QT_PER_S + qt
                nkt = _nkt(qt, mode)
                av = [
                    ps_av.tile([128, 512], F32, tag="av", name=f"av{ch}_{h}")
                    for h in range(HP)
                ]

                def mask_index(kt):
                    if mode == "causal":
                        off = kt - 4 * qt
                        return off if 0 <= off < 4 else None
                    if mode == "general":
                        return qt * KT_PER_S + kt
                    return None

                def emit_scores(sp):
                    """Scores + exp + mask for k-pair sp; returns av sources."""
                    kts = (2 * sp, 2 * sp + 1)
                    ps_s = [
                        ps1024.tile(
                            [128, 1024], F32, tag="ps1024",
                            name=f"s{ch}_{sp}_{h}",
                        )
                        for h in range(HP)
                    ]
                    for i, kt in enumerate(kts):
                        c, ks = b * QT_PER_S + kt // 4, kt % 4
                        for h in range(HP):
                            nc.tensor.matmul(
                                ps_s[h][:, 512 * i : 512 * (i + 1)],
                                kT_sb[
                                    64 * h : 64 * (h + 1),
                                    c,
                                    128 * ks : 128 * (ks + 1),
                                ],
                                qT_sb[64 * h : 64 * (h + 1), ch, :],
                                start=True,
                                stop=True,
                            )
                    p_sb = []
                    for h in range(HP):
                        pt = pp.tile([128, 1024], BF16, tag="p")
                        nc.scalar.activation(
                            pt[:], ps_s[h][:], AF.Exp, scale=float(SCALE)
                        )
                        p_sb.append(pt)
                    av_src = {}
                    for i, kt in enumerate(kts):
                        mi = mask_index(kt)
                        if mi is None:
                            for h in range(HP):
                                av_src[(i, h)] = p_sb[h][
                                    :, 512 * i : 512 * (i + 1)
                                ]
                        else:
                            for h in range(HP):
                                pm = pp.tile([128, 512], BF16, tag="pm", bufs=6)
                                nc.vector.tensor_tensor(
                                    pm[:],
                                    p_sb[h][:, 512 * i : 512 * (i + 1)],
                                    mt_sb[:, mi, :],
                                    mybir.AluOpType.mult,
                                )
                                av_src[(i, h)] = pm[:]
                    return kts, av_src

                def emit_av(state):
                    kts, av_src = state
                    for i, kt in enumerate(kts):
                        for h in range(HP):
                            nc.tensor.matmul(
                                av[h][0:65, :],
                                v_sb[:, b * KT_PER_S + kt, h, 0:65],
                                av_src[(i, h)],
                                start=(kt == 0),
                                stop=(kt == nkt - 1),
                            )

                # software pipeline: scores run two k-pairs ahead of av so
                # the PE has independent work while ACT/DVE produce P.
                pend = []
                for sp in range(nkt // 2):
                    pend.append(emit_scores(sp))
                    if len(pend) > 2:
                        emit_av(pend.pop(0))
                while pend:
                    emit_av(pend.pop(0))
                # epilogue: 1/denom = exp(-ln(denom)) on ACT; PE outer
                # product broadcasts it into av[64:128]; one DVE multiply
                # emits the bf16 A2A payload.
                attnT = misc.tile([128, 512], BF16, tag="attnT", bufs=4)
                for h in range(HP):
                    lnv = misc.tile([1, 512], F32, tag="lnv", bufs=4)
                    nc.scalar.activation(lnv[:], av[h][64:65, :], AF.Ln)
                    rec_bf = misc.tile([1, 512], BF16, tag="recbf", bufs=4)
                    nc.scalar.activation(rec_bf[:], lnv[:], AF.Exp, scale=-1.0)
                    ps_b = ps_misc.tile(
                        [64, 512], F32, tag="psm", name=f"psb{ch}_{h}",
                        padded_shape=[128, 512],
                    )
                    nc.tensor.matmul(
                        ps_b[:], ones64[:], rec_bf[:], start=True, stop=True
                    )
                    recb = misc.tile([64, 512], F32, tag="recb", bufs=4)
                    nc.vector.tensor_copy(recb[:], ps_b[:])
                    nc.vector.tensor_tensor(
                        attnT[64 * h : 64 * (h + 1), :],
                        av[h][0:64, :],
                        recb[:],
                        mybir.AluOpType.mult,
                    )
                if debug_stage == "attn":
                    nc.sync.dma_start(
                        out[64 * ch : 64 * (ch + 1), 0:256].bitcast(BF16),
                        attnT[0:64, :],
                    )
                    nc.sync.dma_start(
                        out[64 * ch : 64 * (ch + 1), 256:512].bitcast(BF16),
                        attnT[64:128, :],
                    )
                    return
                for r in range(NCORES):
                    nc.gpsimd.dma_start(
                        a2a_in[ch][r, :, :],
                        attnT[:, 64 * r : 64 * (r + 1)],
                    )

            def a2a(b, qt):
                ch = b * QT_PER_S + qt
                nc.gpsimd.collective_compute(
                    "AllToAll",
                    mybir.AluOpType.bypass,
                    replica_groups=[list(range(NCORES))],
                    ins=[a2a_in[ch].opt()],
                    outs=[a2a_out[ch].opt()],
                )

            ab_tiles = {}

            def ab_load(qt):
                ab = const.tile([128, 8, 128], BF16, name=f"ab{qt}")
                ab_tiles[qt] = ab
                for b in range(B):
                    nc.sync.dma_start(
                        ab[:, :, 64 * b : 64 * (b + 1)],
                        a2a_out[b * QT_PER_S + qt].rearrange("r p t -> p r t"),
                    )

            def outproj(qt):
                ab = ab_tiles[qt]
                psos = [
                    ps_misc.tile([128, 512], F32, tag="psm", name=f"o{qt}_{h2}")
                    for h2 in range(2)
                ]
                for fb in range(8):
                    for half in range(2):
                        nc.tensor.matmul(
                            psos[half][:],
                            ab[:, fb, :],
                            wo_sb[:, fb, 512 * half : 512 * (half + 1)],
                            start=(fb == 0),
                            stop=(fb == 7),
                        )
                for half in range(2):
                    osb = stage.tile([128, 512], F32, tag="osb", bufs=4)
                    nc.vector.tensor_copy(osb[:], psos[half][:])
                    nc.scalar.dma_start(
                        out[
                            128 * qt : 128 * (qt + 1),
                            512 * half : 512 * (half + 1),
                        ],
                        osb[:],
                    )

            # ---- emission: interleave projection chunks with attention so
            # the PE stream stays dense; per-tile A2As pipeline behind the
            # following attention tiles; outproj(qt) lands as soon as both
            # of its A2As have been given time to complete.
            dbg = debug_stage is not None

            def qkv_chunk_s(c):
                with nc.named_scope(f"qkv{c}"):
                    qkv_chunk(c)

            def attention_s(b, qt):
                with nc.named_scope(f"att{b}{qt}"):
                    attention(b, qt)

            def a2a_s(b, qt):
                with nc.named_scope(f"a2a{b}{qt}"):
                    a2a(b, qt)

            def outproj_s(qt):
                with nc.named_scope(f"oproj{qt}"):
                    outproj(qt)

            qkv_chunk_s(0)
            qkv_chunk_s(4)
            nc.sync.dma_start(wo_sb[:], wo.rearrange("(o p) n -> p o n", p=128))
            attention_s(0, 0)
            if not dbg:
                a2a_s(0, 0)
            qkv_chunk_s(1)
            attention_s(1, 0)
            if not dbg:
                a2a_s(1, 0)
                ab_load(0)
            qkv_chunk_s(5)
            attention_s(0, 1)
            if not dbg:
                a2a_s(0, 1)
            qkv_chunk_s(2)
            attention_s(1, 1)
            if not dbg:
                a2a_s(1, 1)
                ab_load(1)
            qkv_chunk_s(6)
            attention_s(0, 2)
            if not dbg:
                a2a_s(0, 2)
            qkv_chunk_s(3)
            attention_s(1, 2)
            if not dbg:
                a2a_s(1, 2)
                ab_load(2)
            qkv_chunk_s(7)
            attention_s(0, 3)
            if not dbg:
                a2a_s(0, 3)
            attention_s(1, 3)
            if not dbg:
                a2a_s(1, 3)
                ab_load(3)
                outproj_s(0)
                outproj_s(1)
                outproj_s(2)
                outproj_s(3)
            _stk.close()
    _split_waits(nc)

    # Encode a hash of the BIR into the shape of an unused dummy input so
    # the HLO (and therefore the NEFF cache key) changes with the kernel.
    import hashlib

    hv = int.from_bytes(
        hashlib.sha256(nc.to_json_bytes()).digest()[:4], "little"
    )
    nonce_shape = [hv % 1021 + 1, (hv // 1021) % 1021 + 1]
    nc.dram_tensor("nonce", nonce_shape, F32, kind="ExternalInput")
    nc._nonce_shape = nonce_shape
    return nc


_BUILD_CACHE = {}


def _get_nc(mode, n_mask_tiles, debug_stage=None):
    key = (mode, n_mask_tiles, debug_stage)
    if key not in _BUILD_CACHE:
        _BUILD_CACHE[key] = build(mode, n_mask_tiles, debug_stage)
    return _BUILD_CACHE[key]


def kernel(x, Wqkv, Wo, mask):
    x = np.asarray(x)
    Wqkv = np.asarray(Wqkv)
    Wo = np.asarray(Wo)
    mask = np.asarray(mask)

    m2 = mask.reshape(S, S)
    if np.array_equal(m2, np.tril(np.ones((S, S), bool))):
        mode = "causal"
    elif m2.all():
        mode = "full"
    else:
        mode = "general"

    xT = np.ascontiguousarray(x.reshape(T, DM).T).astype(ml_dtypes.bfloat16)
    w4 = Wqkv.reshape(DM, H, 3, D)

    if mode == "causal":
        qq = np.arange(512)[None, :]
        kk = np.arange(128)[:, None]
        mts = np.stack(
            [(qq - kk >= 128 * o) for o in range(4)]
        ).astype(ml_dtypes.bfloat16)
        n_mask_tiles = 4
    elif mode == "general":
        tiles = []
        for qt in range(QT_PER_S):
            for kt in range(KT_PER_S):
                sub = m2[512 * qt : 512 * (qt + 1), 128 * kt : 128 * (kt + 1)]
                tiles.append(sub.T)
        mts = np.stack(tiles).astype(ml_dtypes.bfloat16)
        n_mask_tiles = len(tiles)
    else:
        mts = None
        n_mask_tiles = 0

    nc = _get_nc(mode, n_mask_tiles)

    in_maps = []
    for j in range(NCORES):
        hs = slice(HP * j, HP * (j + 1))
        im = {
            "xT": xT,
            "wq": np.ascontiguousarray(
                w4[:, hs, 0, :].reshape(DM, HP * D)
            ).astype(ml_dtypes.bfloat16),
            "wk": np.ascontiguousarray(
                w4[:, hs, 1, :].reshape(DM, HP * D)
            ).astype(ml_dtypes.bfloat16),
            "wv": np.ascontiguousarray(
                w4[:, hs, 2, :].reshape(DM, HP * D)
            ).astype(ml_dtypes.bfloat16),
            "wo": Wo.astype(ml_dtypes.bfloat16),
            "nonce": np.zeros(nc._nonce_shape, np.float32),
        }
        if n_mask_tiles:
            im["mt"] = mts
        in_maps.append(im)

    res = run_bass_kernel_spmd(nc, in_maps, list(range(NCORES)))
    # core j's output rows: [128*qt + 64*b + i] = batch b token
    # 512*qt + 64*j + i.
    full = np.empty((B, S, DM), np.float32)
    for j in range(NCORES):
        o = res.results[j]["out"]
        for qt in range(QT_PER_S):
            for b in range(B):
                full[b, 512 * qt + 64 * j : 512 * qt + 64 * (j + 1), :] = o[
                    128 * qt + 64 * b : 128 * qt + 64 * (b + 1)
                ]
    return full


if __name__ == "__main__":
    rng = np.random.default_rng(0)
    x = rng.standard_normal((B, S, DM), dtype=np.float32)
    Wqkv = rng.standard_normal((DM, 3 * H * D), dtype=np.float32) * DM**-0.5
    Wo = rng.standard_normal((H * D, DM), dtype=np.float32) * (H * D) ** -0.5
    mask = np.tril(np.ones((S, S), bool))[None, None]
    out = kernel(x=x, Wqkv=Wqkv, Wo=Wo, mask=mask)
    print(out.shape, out.dtype)

